# revision 3
# baseline (speedup 1.0000x reference)
"""Causal multi-head self-attention with RoPE on 8 TRN2 NeuronCores.

Sharding: data-parallel over batch (4) x tensor-parallel over heads (16 -> 2
groups of 8).  Core c handles batch c//2, head group c%2.  Each core computes
its 8 heads' attention and a partial O-projection (512 of the 1024 contraction
dims); the host sums the two partials per batch element.
"""

import os
import sys

import numpy as np

if "/opt/trn_rl_repo" not in sys.path:
    sys.path.insert(0, "/opt/trn_rl_repo")

D_MODEL = 1024
NUM_HEADS = 16
THETA = 10000.0
B, S = 4, 2048
DK = 64
HALF = DK // 2
P = 128
N_CORES = 8
HPC = 8                 # heads per core
DOUT = HPC * DK         # 512 per-core projected dims
KT = D_MODEL // P       # 8 contraction tiles
NSEQ = S // P           # 16 seq tiles of 128
NQB = S // 512          # 4 query blocks of 512
SCALE = 1.0 / np.sqrt(DK)

_CACHE = {}


def _build():
    """Build + compile the per-core Bass module (same program on all cores)."""
    import concourse.bass as bass
    import concourse.bacc as bacc
    import concourse.tile as tile
    import concourse.mybir as mybir
    from contextlib import ExitStack

    f32 = mybir.dt.float32
    bf16 = mybir.dt.bfloat16
    Exp = mybir.ActivationFunctionType.Exp

    nc = bacc.Bacc("TRN2", target_bir_lowering=False, debug=False,
                   enable_asserts=False, num_devices=N_CORES)

    xT = nc.dram_tensor("xT", [D_MODEL, S], bf16, kind="ExternalInput")
    wq = nc.dram_tensor("wq", [D_MODEL, DOUT], bf16, kind="ExternalInput")
    wk = nc.dram_tensor("wk", [D_MODEL, DOUT], bf16, kind="ExternalInput")
    wv = nc.dram_tensor("wv", [D_MODEL, DOUT], bf16, kind="ExternalInput")
    wo = nc.dram_tensor("wo", [DOUT, D_MODEL], bf16, kind="ExternalInput")
    cosn = nc.dram_tensor("cosn", [S, DK], f32, kind="ExternalInput")
    sinn = nc.dram_tensor("sinn", [S, DK], f32, kind="ExternalInput")
    maskt = nc.dram_tensor("maskt", [P, 4 * 512], bf16, kind="ExternalInput")
    ident = nc.dram_tensor("ident", [P, P], bf16, kind="ExternalInput")
    out = nc.dram_tensor("out", [S, D_MODEL], f32, kind="ExternalOutput")

    def rep8(ap):
        # replicate a [128, 64] tile 8x along free dim -> logical [128, 512]
        return bass.AP(tensor=ap.tensor, offset=ap.offset,
                       ap=[ap.ap[0], [0, HPC], [1, DK]])

    def pairswap(ap):
        # free-dim pair swap of a [128, 512] tile: (0,1,2,3,..)->(1,0,3,2,..)
        return bass.AP(tensor=ap.tensor, offset=ap.offset + 1,
                       ap=[ap.ap[0], [2, 256], [-1, 2]])

    with tile.TileContext(nc) as tc, ExitStack() as top:
        persist = top.enter_context(tc.tile_pool(name="persist", bufs=1))

        # ---- persistent SBUF arrays ----
        x_sb = []
        for k in range(KT):
            t = persist.tile([P, S], bf16, tag=f"x{k}", name=f"x{k}")
            nc.sync.dma_start(out=t, in_=xT[k * P:(k + 1) * P, :])
            x_sb.append(t)
        w_sb = {}
        for nm, dram in (("wq", wq), ("wk", wk), ("wv", wv)):
            w_sb[nm] = []
            for k in range(KT):
                t = persist.tile([P, DOUT], bf16, tag=f"{nm}{k}", name=f"{nm}{k}")
                nc.sync.dma_start(out=t, in_=dram[k * P:(k + 1) * P, :])
                w_sb[nm].append(t)
        wo_sb = []
        for k in range(DOUT // P):
            t = persist.tile([P, D_MODEL], bf16, tag=f"wo{k}", name=f"wo{k}")
            nc.sync.dma_start(out=t, in_=wo[k * P:(k + 1) * P, :])
            wo_sb.append(t)
        cos_sb, sin_sb = [], []
        for m in range(NSEQ):
            tcs = persist.tile([P, DK], f32, tag=f"cos{m}", name=f"cos{m}")
            nc.sync.dma_start(out=tcs, in_=cosn[m * P:(m + 1) * P, :])
            cos_sb.append(tcs)
            tsn = persist.tile([P, DK], f32, tag=f"sin{m}", name=f"sin{m}")
            nc.sync.dma_start(out=tsn, in_=sinn[m * P:(m + 1) * P, :])
            sin_sb.append(tsn)
        mask_sb = persist.tile([P, 4 * 512], bf16, tag="mask", name="mask")
        nc.sync.dma_start(out=mask_sb, in_=maskt[:, :])
        id_sb = persist.tile([P, P], bf16, tag="ident", name="ident")
        nc.sync.dma_start(out=id_sb, in_=ident[:, :])

        # outputs of phase A
        qt_sb = [persist.tile([P, S], bf16, tag=f"qt{d}", name=f"qt{d}")
                 for d in range(4)]
        kt_sb = [persist.tile([P, S], bf16, tag=f"kt{d}", name=f"kt{d}")
                 for d in range(4)]
        v_sb = [persist.tile([P, HPC * (DK + 1)], bf16, tag=f"v{t}", name=f"v{t}")
                for t in range(NSEQ)]
        ot_sb = [persist.tile([P, S], bf16, tag=f"ot{d}", name=f"ot{d}")
                 for d in range(4)]

        # ---- phase A: projections + rope + transpose ----
        with ExitStack() as pa:
            qk_ps = pa.enter_context(
                tc.tile_pool(name="qk_ps", bufs=2, space="PSUM"))
            v_ps = pa.enter_context(
                tc.tile_pool(name="v_ps", bufs=2, space="PSUM"))
            tr_ps = pa.enter_context(
                tc.tile_pool(name="tr_ps", bufs=4, space="PSUM"))
            ropet = pa.enter_context(tc.tile_pool(name="ropet", bufs=2))
            natp = pa.enter_context(tc.tile_pool(name="natp", bufs=4))

            for nm, dst, copy_eng in (("wq", qt_sb, "v"), ("wk", kt_sb, "s")):
                trt = [None] * 4
                for m in range(NSEQ):
                    ps = qk_ps.tile([P, DOUT], f32, tag="qk")
                    for k in range(KT):
                        nc.tensor.matmul(ps, x_sb[k][:, m * P:(m + 1) * P],
                                         w_sb[nm][k], start=(k == 0),
                                         stop=(k == KT - 1))
                    t1 = ropet.tile([P, DOUT], f32, tag="rt1", name="rt1")
                    t2 = ropet.tile([P, DOUT], f32, tag="rt2", name="rt2")
                    nc.vector.tensor_mul(t1, ps, rep8(cos_sb[m]))
                    nc.vector.tensor_mul(t2, pairswap(ps), rep8(sin_sb[m]))
                    nat = natp.tile([P, DOUT], bf16, tag="nat", name="nat")
                    nc.vector.tensor_add(nat, t1, t2)
                    for d in range(4):
                        if m % 4 == 0:
                            trt[d] = tr_ps.tile([P, 512], bf16, tag="tr",
                                                name=f"tr{d}")
                        nc.tensor.transpose(
                            trt[d][:, (m % 4) * P:(m % 4 + 1) * P],
                            nat[:, d * P:(d + 1) * P], id_sb)
                    if m % 4 == 3:
                        g = m // 4
                        for d in range(4):
                            if copy_eng == "v":
                                nc.vector.tensor_copy(
                                    dst[d][:, g * 512:(g + 1) * 512], trt[d])
                            else:
                                nc.scalar.copy(
                                    dst[d][:, g * 512:(g + 1) * 512], trt[d])

            for m in range(NSEQ):
                ps = v_ps.tile([P, DOUT], f32, tag="v")
                for k in range(KT):
                    nc.tensor.matmul(ps, x_sb[k][:, m * P:(m + 1) * P],
                                     w_sb["wv"][k], start=(k == 0),
                                     stop=(k == KT - 1))
                vt = v_sb[m]
                ones_ap = bass.AP(tensor=vt.tensor, offset=vt.offset + DK,
                                  ap=[vt.ap[0], [DK + 1, HPC]])
                nc.gpsimd.memset(ones_ap, 1.0)
                vcols = bass.AP(tensor=vt.tensor, offset=vt.offset,
                                ap=[vt.ap[0], [DK + 1, HPC], [1, DK]])
                nc.scalar.copy(vcols, ps)

        # ---- phase B: attention per head ----
        with ExitStack() as pb:
            sc_ps = pb.enter_context(
                tc.tile_pool(name="sc_ps", bufs=1, space="PSUM"))
            av_ps = pb.enter_context(
                tc.tile_pool(name="av_ps", bufs=1, space="PSUM"))
            ptp = pb.enter_context(tc.tile_pool(name="ptp", bufs=2))
            rcpp = pb.enter_context(tc.tile_pool(name="rcpp", bufs=2))
            rmatp = pb.enter_context(tc.tile_pool(name="rmatp", bufs=2))

            for h in range(HPC):
                db, po = h // 2, (h % 2) * DK
                av = av_ps.tile([DK + 1, S], f32, tag="av")
                for t in range(NSEQ):
                    qb0 = t // 4
                    span = slice(qb0 * 512, S)
                    sc = sc_ps.tile([P, S], f32, tag="sc")
                    for qb in range(qb0, NQB):
                        nc.tensor.matmul(
                            sc[:, qb * 512:(qb + 1) * 512],
                            kt_sb[db][po:po + DK, t * P:(t + 1) * P],
                            qt_sb[db][po:po + DK, qb * 512:(qb + 1) * 512],
                            start=True, stop=True)
                    pt = ptp.tile([P, S], bf16, tag="pt", name="pt")
                    nc.scalar.activation(pt[:, span], sc[:, span], Exp)
                    v = t % 4
                    nc.vector.tensor_mul(
                        pt[:, qb0 * 512:(qb0 + 1) * 512],
                        pt[:, qb0 * 512:(qb0 + 1) * 512],
                        mask_sb[:, v * 512:(v + 1) * 512])
                    for qb in range(qb0, NQB):
                        nc.tensor.matmul(
                            av[:, qb * 512:(qb + 1) * 512],
                            v_sb[t][:, h * (DK + 1):(h + 1) * (DK + 1)],
                            pt[:, qb * 512:(qb + 1) * 512],
                            start=(t == 0), stop=(t == 4 * qb + 3))
                for qb in range(NQB):
                    cols = slice(qb * 512, (qb + 1) * 512)
                    rcp = rcpp.tile([1, 512], f32, tag="rcp", name="rcp")
                    nc.vector.reciprocal(rcp, av[DK:DK + 1, cols])
                    rmat = rmatp.tile([DK, 512], f32, tag="rmat", name="rmat")
                    nc.gpsimd.partition_broadcast(rmat, rcp, channels=DK)
                    nc.vector.tensor_mul(ot_sb[db][po:po + DK, cols],
                                         av[0:DK, cols], rmat)

        # ---- phase C: output projection ----
        with ExitStack() as pc:
            op_ps = pc.enter_context(
                tc.tile_pool(name="op_ps", bufs=2, space="PSUM"))
            ostg = pc.enter_context(tc.tile_pool(name="ostg", bufs=3))
            for m in range(NSEQ):
                for nb in range(2):
                    ps = op_ps.tile([P, 512], f32, tag="op")
                    for k in range(4):
                        nc.tensor.matmul(
                            ps, ot_sb[k][:, m * P:(m + 1) * P],
                            wo_sb[k][:, nb * 512:(nb + 1) * 512],
                            start=(k == 0), stop=(k == 3))
                    og = ostg.tile([P, 512], f32, tag="og", name="og")
                    nc.scalar.copy(og, ps)
                    nc.sync.dma_start(
                        out=out[m * P:(m + 1) * P, nb * 512:(nb + 1) * 512],
                        in_=og)

    nc.compile()
    return nc


def _get_nc():
    if "nc" not in _CACHE:
        _CACHE["nc"] = _build()
    return _CACHE["nc"]


def _prep_core_inputs(q_proj_weight, k_proj_weight, v_proj_weight,
                      o_proj_weight, in_features, token_positions):
    """Host-side sharding: returns the list of 8 per-core input dicts."""
    import ml_dtypes
    bf = ml_dtypes.bfloat16

    x = np.asarray(in_features, np.float32)
    wqf = np.asarray(q_proj_weight, np.float32)
    wkf = np.asarray(k_proj_weight, np.float32)
    wvf = np.asarray(v_proj_weight, np.float32)
    wof = np.asarray(o_proj_weight, np.float32)
    tp = np.asarray(token_positions).astype(np.float64)

    inv = 1.0 / (THETA ** (np.arange(HALF, dtype=np.float64) / HALF))
    fr = tp[:, None] * inv[None, :]                       # [S, 32]
    cosn = np.repeat(np.cos(fr), 2, axis=1).astype(np.float32)  # [S, 64]
    sg = np.tile(np.array([-1.0, 1.0]), HALF)[None, :]
    sinn = (np.repeat(np.sin(fr), 2, axis=1) * sg).astype(np.float32)

    kv = np.arange(P)[:, None]
    qc = np.arange(512)[None, :]
    maskt = np.concatenate(
        [(qc >= 128 * v + kv) for v in range(4)], axis=1).astype(bf)

    identity = np.eye(P, dtype=bf)

    in_maps = []
    for c in range(N_CORES):
        b, hg = c // 2, c % 2
        rows = slice(hg * DOUT, (hg + 1) * DOUT)
        wv_s = wvf[rows].T.astype(bf)                      # [1024, 512]
        in_maps.append({
            "xT": np.ascontiguousarray(x[b].T).astype(bf),
            "wq": np.ascontiguousarray((wqf[rows] * SCALE).T).astype(bf),
            "wk": np.ascontiguousarray(wkf[rows].T).astype(bf),
            "wv": np.ascontiguousarray(wv_s),
            "wo": np.ascontiguousarray(wof[:, rows].T).astype(bf),
            "cosn": cosn,
            "sinn": sinn,
            "maskt": maskt,
            "ident": identity,
        })
    return in_maps


def kernel(q_proj_weight, k_proj_weight, v_proj_weight, o_proj_weight,
           in_features, token_positions):
    from concourse.bass_utils import run_bass_kernel_spmd

    nc = _get_nc()
    in_maps = _prep_core_inputs(q_proj_weight, k_proj_weight, v_proj_weight,
                                o_proj_weight, in_features, token_positions)
    trace = bool(int(os.environ.get("KBENCH_TRACE", "0")))
    res = run_bass_kernel_spmd(nc, in_maps, list(range(N_CORES)), trace=trace)
    _CACHE["last_results"] = res
    if res.exec_time_ns is not None:
        _CACHE["exec_time_ns"] = res.exec_time_ns

    outp = np.empty((B, S, D_MODEL), np.float32)
    for b in range(B):
        outp[b] = res.results[2 * b]["out"] + res.results[2 * b + 1]["out"]
    return outp
